# revision 2
# baseline (speedup 1.0000x reference)
"""Multi-level ROI Align (FPN pooler, 4 levels summed) on 8 Trainium2 cores.

Strategy: shard ROIs across cores (core k: batch k//4, 128 ROIs). All gather
indices and bilinear weights are computed on host from `boxes`; the device
kernel does the heavy lifting: HBM pixel gathers (dma_gather) + weighted
scatter-reduction into 7x7 bins via PSUM-accumulating matmuls.

Per ROI, per level:
  out[bin, c] = sum_j W[j, bin] * G[j, c]
where G rows are gathered pixel vectors (C=256) and W is sparse (built on
device as fixed_pattern * per-partition scalar for L0/L1, host-baked dense
for the region-gathered L2/L3).

L0 uses 3-pixel elements addressed at even-pixel granularity (idx = flat//2)
to fit the int16 index range (200*200 = 40000 > 32767).
"""
import sys
import numpy as np

sys.path.insert(0, '/opt/trn_rl_repo')

POOLED = 7
SAMP = 2
NBIN = 49
C = 256
IMG = 800.0

# per level: H, W, scale, mode
#   mode 'tri': 3-px elems, idx=flat//2, NJ j's with 3 weight slots
#   mode 'px' : 1-px elems, corner gathers
#   mode 'reg': 1-px elems, bounding-region pixels, host-baked lhsT
L0 = dict(H=200, W=200, scale=0.25, mode='tri', NJ=512, REAL=392, NCH=4)
L1 = dict(H=100, W=100, scale=0.125, mode='px', NJ=896, REAL=784, NCH=7)
L2 = dict(H=50, W=50, scale=0.0625, mode='reg', NJ=384, REAL=324, NCH=3, WREG=18)
L3 = dict(H=25, W=25, scale=0.03125, mode='reg', NJ=128, REAL=100, NCH=1, WREG=10)
LEVELS = [L0, L1, L2, L3]

NROI_CORE = 128     # ROIs per core
NGRP = 64           # groups of 2 ROIs
GRP = 2

# padded flat pixel counts of the feature buffers
F0_ROWS = 40004     # covers 3-px elem overrun
F1_ROWS = 10000
F2_ROWS = 3400      # covers region overrun (y,x up to 66)
F3_ROWS = 900       # covers region overrun (y,x up to 33)

# const fp32 column layout (per partition)
PAT0_OFF = 0                       # [4, 49]
PAT1_OFF = PAT0_OFF + 4 * NBIN     # [7, 49]
WCOL0_OFF = PAT1_OFF + 7 * NBIN    # [128 roi * 12]
WCOL1_OFF = WCOL0_OFF + NROI_CORE * 12   # [128 roi * 7]
ID_OFF = WCOL1_OFF + NROI_CORE * 7       # [49]
CST_COLS = ID_OFF + NBIN

# idx int16 column layout (per partition), per 2-ROI group
IC0, IC1, IC2, IC3 = 64, 112, 48, 16     # cols per group per level
IDX0_OFF = 0
IDX1_OFF = IDX0_OFF + NGRP * IC0
IDX2_OFF = IDX1_OFF + NGRP * IC1
IDX3_OFF = IDX2_OFF + NGRP * IC2
IDX_COLS = IDX3_OFF + NGRP * IC3

_MODULE_CACHE = {}


def _sample_meta(boxes_b, H, W, scale):
    """Per-ROI sample geometry in fp32, matching reference op order.
    boxes_b: [N, 4] fp32. Returns dict of [N,7,2] arrays."""
    f = np.float32
    b = boxes_b.astype(np.float32)
    x1 = b[:, 0] * f(scale)
    y1 = b[:, 1] * f(scale)
    x2 = b[:, 2] * f(scale)
    y2 = b[:, 3] * f(scale)
    rw = np.maximum(x2 - x1, f(1.0))
    rh = np.maximum(y2 - y1, f(1.0))
    bw = rw / f(POOLED)
    bh = rh / f(POOLED)
    g = (np.arange(POOLED, dtype=np.float32)[:, None]
         + (np.arange(SAMP, dtype=np.float32)[None, :] + f(0.5)) / f(SAMP))
    y = y1[:, None, None] + g[None] * bh[:, None, None]   # [N,7,2]
    x = x1[:, None, None] + g[None] * bw[:, None, None]
    masky = ((y >= f(-1.0)) & (y <= f(H))).astype(np.float32)
    maskx = ((x >= f(-1.0)) & (x <= f(W))).astype(np.float32)
    yc = np.clip(y, f(0.0), f(H - 1))
    xc = np.clip(x, f(0.0), f(W - 1))
    yl = np.floor(yc).astype(np.int64)
    xl = np.floor(xc).astype(np.int64)
    yh = np.minimum(yl + 1, H - 1)
    xh = np.minimum(xl + 1, W - 1)
    ly = (yc - yl.astype(np.float32)).astype(np.float32)
    lx = (xc - xl.astype(np.float32)).astype(np.float32)
    hy = (f(1.0) - ly).astype(np.float32)
    hx = (f(1.0) - lx).astype(np.float32)
    return dict(yl=yl, yh=yh, xl=xl, xh=xh, ly=ly, lx=lx, hy=hy, hx=hx,
                masky=masky, maskx=maskx, x=x, y=y)


def _build_tri(meta, lv):
    """L0: j = (row_sel, py, sy, px, sx) -> 392 3-px elems, 3 slot weights.
    Returns idx [N, NJ] int64, w [N, NJ, 3] fp32."""
    N = meta['yl'].shape[0]
    W = lv['W']
    NJ, REAL = lv['NJ'], lv['REAL']
    rows = np.stack([meta['yl'], meta['yh']], axis=1)          # [N,2,7,2] (rs)
    wys = np.stack([meta['hy'], meta['ly']], axis=1)           # [N,2,7,2]
    m = (meta['masky'][:, :, :, None, None] * meta['maskx'][:, None, None, :, :])  # [N,7,2,7,2]
    # broadcast to [N, rs, py, sy, px, sx]
    row = np.broadcast_to(rows[:, :, :, :, None, None], (N, 2, 7, 2, 7, 2))
    wy = np.broadcast_to(wys[:, :, :, :, None, None], (N, 2, 7, 2, 7, 2)).astype(np.float32)
    xl = np.broadcast_to(meta['xl'][:, None, None, None, :, :], (N, 2, 7, 2, 7, 2))
    hx = np.broadcast_to(meta['hx'][:, None, None, None, :, :], (N, 2, 7, 2, 7, 2)).astype(np.float32)
    lx = np.broadcast_to(meta['lx'][:, None, None, None, :, :], (N, 2, 7, 2, 7, 2)).astype(np.float32)
    mm = np.broadcast_to(m[:, None], (N, 2, 7, 2, 7, 2)).astype(np.float32)
    flat = row * W + xl
    idx = (flat >> 1).reshape(N, REAL)
    r = (flat & 1).astype(np.float32).reshape(N, REAL)
    wl = (wy * hx * mm * np.float32(0.25)).reshape(N, REAL)
    wh = (wy * lx * mm * np.float32(0.25)).reshape(N, REAL)
    w = np.zeros((N, NJ, 3), np.float32)
    w[:, :REAL, 0] = wl * (1 - r)
    w[:, :REAL, 1] = wl * r + wh * (1 - r)
    w[:, :REAL, 2] = wh * r
    idx_full = np.zeros((N, NJ), np.int64)
    idx_full[:, :REAL] = idx
    return idx_full, w


def _build_px(meta, lv):
    """L1: j = (row_sel, col_sel, py, sy, px, sx) -> 784 1-px corner gathers.
    Returns idx [N, NJ] int64, w [N, NJ] fp32."""
    N = meta['yl'].shape[0]
    W = lv['W']
    NJ, REAL = lv['NJ'], lv['REAL']
    rows = np.stack([meta['yl'], meta['yh']], axis=1)   # [N,2(rs),7,2]
    wys = np.stack([meta['hy'], meta['ly']], axis=1)
    cols = np.stack([meta['xl'], meta['xh']], axis=1)   # [N,2(cs),7,2]
    wxs = np.stack([meta['hx'], meta['lx']], axis=1)
    m = (meta['masky'][:, :, :, None, None] * meta['maskx'][:, None, None, :, :])
    row = np.broadcast_to(rows[:, :, None, :, :, None, None], (N, 2, 2, 7, 2, 7, 2))
    wy = np.broadcast_to(wys[:, :, None, :, :, None, None], (N, 2, 2, 7, 2, 7, 2)).astype(np.float32)
    col = np.broadcast_to(cols[:, None, :, None, None, :, :], (N, 2, 2, 7, 2, 7, 2))
    wx = np.broadcast_to(wxs[:, None, :, None, None, :, :], (N, 2, 2, 7, 2, 7, 2)).astype(np.float32)
    mm = np.broadcast_to(m[:, None, None], (N, 2, 2, 7, 2, 7, 2)).astype(np.float32)
    idx = (row * W + col).reshape(N, REAL)
    w = (wy * wx * mm * np.float32(0.25)).reshape(N, REAL)
    idx_full = np.zeros((N, NJ), np.int64)
    w_full = np.zeros((N, NJ), np.float32)
    idx_full[:, :REAL] = idx
    w_full[:, :REAL] = w
    return idx_full, w_full


def _build_reg(meta, lv):
    """L2/L3: bounding-region pixels + separable host-baked weights.
    Returns idx [N, NJ] int64, lhsT [N, NJ, 49] fp32."""
    N = meta['yl'].shape[0]
    H, W, WREG = lv['H'], lv['W'], lv['WREG']
    NJ, REAL = lv['NJ'], lv['REAL']
    f = np.float32
    y_base = np.floor(np.clip(meta['y'].reshape(N, -1).min(1), 0.0, H - 1)).astype(np.int64)
    x_base = np.floor(np.clip(meta['x'].reshape(N, -1).min(1), 0.0, W - 1)).astype(np.int64)
    # WY [N, WREG, 7], WX [N, WREG, 7]
    WY = np.zeros((N, WREG, POOLED), np.float32)
    WX = np.zeros((N, WREG, POOLED), np.float32)
    ridx = np.arange(N)[:, None, None]
    pidx = np.broadcast_to(np.arange(POOLED)[None, :, None], (N, POOLED, SAMP))
    np.add.at(WY, (ridx, meta['yl'] - y_base[:, None, None], pidx),
              (f(0.5) * meta['hy'] * meta['masky']).astype(np.float32))
    np.add.at(WY, (ridx, meta['yh'] - y_base[:, None, None], pidx),
              (f(0.5) * meta['ly'] * meta['masky']).astype(np.float32))
    np.add.at(WX, (ridx, meta['xl'] - x_base[:, None, None], pidx),
              (f(0.5) * meta['hx'] * meta['maskx']).astype(np.float32))
    np.add.at(WX, (ridx, meta['xh'] - x_base[:, None, None], pidx),
              (f(0.5) * meta['lx'] * meta['maskx']).astype(np.float32))
    lhsT = np.einsum('nap,nbq->nabpq', WY, WX).reshape(N, REAL, NBIN)
    dy = np.arange(WREG)
    idx = ((y_base[:, None, None] + dy[None, :, None]) * W
           + x_base[:, None, None] + dy[None, None, :]).reshape(N, REAL)
    idx_full = np.zeros((N, NJ), np.int64)
    lhsT_full = np.zeros((N, NJ, NBIN), np.float32)
    idx_full[:, :REAL] = idx
    lhsT_full[:, :REAL] = lhsT
    return idx_full, lhsT_full


def _pack_idx(jlists):
    """Pack concatenated per-group idx list [NJ_total] -> [128, NJ_total//16]
    int16 wrapped in 16 partitions, replicated 8x."""
    jl = np.asarray(jlists)
    n = jl.shape[-1]
    arr = jl.reshape(*jl.shape[:-1], n // 16, 16)   # [..., col, p]
    arr = np.swapaxes(arr, -1, -2)                  # [..., p(16), col]
    arr = np.broadcast_to(arr[..., None, :, :],
                          (*jl.shape[:-1], 8, 16, n // 16))
    return arr.reshape(*jl.shape[:-1], 128, n // 16).astype(np.int16)


def _bin_pattern(mode, NCH, REAL):
    """Fixed j->bin one-hot pattern [128, NCH, 49] for 'tri'/'px' j order."""
    NJ = NCH * 128
    j = np.arange(NJ)
    if mode == 'tri':
        # j = ((((rs*7+py)*2+sy)*7+px)*2+sx)
        px = (j // 2) % 7
        py = (j // (2 * 7 * 2)) % 7
    else:
        # j = (((((rs*2+cs)*7+py)*2+sy)*7+px)*2+sx)
        px = (j // 2) % 7
        py = (j // (2 * 7 * 2)) % 7
    bins = py * 7 + px
    pat = np.zeros((NJ, NBIN), np.float32)
    valid = j < REAL
    pat[np.arange(NJ)[valid], bins[valid]] = 1.0
    return pat.reshape(NCH, 128, NBIN).transpose(1, 0, 2)   # [128, NCH, 49]


def _host_prepare(x0, x1, x2, x3, boxes):
    """Build all per-core input tensors. Returns list of 8 dicts."""
    B = boxes.shape[0]
    feats = []
    for arr, lv, rows in ((x0, L0, F0_ROWS), (x1, L1, F1_ROWS),
                          (x2, L2, F2_ROWS), (x3, L3, F3_ROWS)):
        f = np.zeros((B, rows, C), np.float32)
        hw = lv['H'] * lv['W']
        f[:, :hw] = np.ascontiguousarray(
            np.transpose(np.asarray(arr, np.float32), (0, 2, 3, 1))).reshape(B, hw, C)
        feats.append(f)

    per_batch = []
    for b in range(B):
        bb = np.asarray(boxes[b], np.float32)
        m0 = _sample_meta(bb, L0['H'], L0['W'], L0['scale'])
        m1 = _sample_meta(bb, L1['H'], L1['W'], L1['scale'])
        m2 = _sample_meta(bb, L2['H'], L2['W'], L2['scale'])
        m3 = _sample_meta(bb, L3['H'], L3['W'], L3['scale'])
        idx0, w0 = _build_tri(m0, L0)
        idx1, w1 = _build_px(m1, L1)
        idx2, lt2 = _build_reg(m2, L2)
        idx3, lt3 = _build_reg(m3, L3)
        per_batch.append((idx0, w0, idx1, w1, idx2, lt2, idx3, lt3))

    pat0 = _bin_pattern('tri', L0['NCH'], L0['REAL'])
    pat1 = _bin_pattern('px', L1['NCH'], L1['REAL'])

    in_maps = []
    for k in range(8):
        b = k // 4
        s = (k % 4) * NROI_CORE
        idx0, w0, idx1, w1, idx2, lt2, idx3, lt3 = per_batch[b]
        sl = slice(s, s + NROI_CORE)

        cst = np.zeros((128, CST_COLS), np.float32)
        cst[:, PAT0_OFF:PAT0_OFF + 4 * NBIN] = pat0.reshape(128, -1)
        cst[:, PAT1_OFF:PAT1_OFF + 7 * NBIN] = pat1.reshape(128, -1)
        # wcol0 [128, roi*12]: col roi*12 + c*3 + slot = w0[roi, c*128+p, slot]
        wc0 = w0[sl].reshape(NROI_CORE, L0['NCH'], 128, 3)   # [roi,c,p,s]
        cst[:, WCOL0_OFF:WCOL0_OFF + NROI_CORE * 12] = (
            wc0.transpose(2, 0, 1, 3).reshape(128, -1))
        wc1 = w1[sl].reshape(NROI_CORE, L1['NCH'], 128)      # [roi,c,p]
        cst[:, WCOL1_OFF:WCOL1_OFF + NROI_CORE * 7] = (
            wc1.transpose(2, 0, 1).reshape(128, -1))
        cst[:NBIN, ID_OFF:ID_OFF + NBIN] = np.eye(NBIN, dtype=np.float32)

        idxs = np.zeros((128, IDX_COLS), np.int16)
        idxs[:, IDX0_OFF:IDX0_OFF + NGRP * IC0] = _pack_idx(
            idx0[sl].reshape(NGRP, GRP * L0['NJ'])).transpose(1, 0, 2).reshape(128, -1)
        idxs[:, IDX1_OFF:IDX1_OFF + NGRP * IC1] = _pack_idx(
            idx1[sl].reshape(NGRP, GRP * L1['NJ'])).transpose(1, 0, 2).reshape(128, -1)
        idxs[:, IDX2_OFF:IDX2_OFF + NGRP * IC2] = _pack_idx(
            idx2[sl].reshape(NGRP, GRP * L2['NJ'])).transpose(1, 0, 2).reshape(128, -1)
        idxs[:, IDX3_OFF:IDX3_OFF + NGRP * IC3] = _pack_idx(
            idx3[sl].reshape(NGRP, GRP * L3['NJ'])).transpose(1, 0, 2).reshape(128, -1)

        # lhsT k-major: lt2 [roi, NJ(=3*128), 49] -> [roi, 128, 3, 49]
        lt2k = np.ascontiguousarray(
            lt2[sl].reshape(NROI_CORE, L2['NCH'], 128, NBIN).transpose(0, 2, 1, 3))
        lt3k = np.ascontiguousarray(lt3[sl].reshape(NROI_CORE, 128, NBIN))

        in_maps.append({
            "f0": feats[0][b], "f1": feats[1][b],
            "f2": feats[2][b], "f3": feats[3][b],
            "cst": cst, "idxs": idxs, "lt2": lt2k, "lt3": lt3k,
        })
    return in_maps


def _build_module():
    from concourse import bacc, tile
    from concourse.bass import mybir
    import concourse.bass as bass_mod

    F32 = mybir.dt.float32
    I16 = mybir.dt.int16
    AP = bass_mod.AP

    nc = bacc.Bacc(None, target_bir_lowering=False)
    f0 = nc.dram_tensor("f0", [F0_ROWS, C], F32, kind="ExternalInput")
    f1 = nc.dram_tensor("f1", [F1_ROWS, C], F32, kind="ExternalInput")
    f2 = nc.dram_tensor("f2", [F2_ROWS, C], F32, kind="ExternalInput")
    f3 = nc.dram_tensor("f3", [F3_ROWS, C], F32, kind="ExternalInput")
    cst = nc.dram_tensor("cst", [128, CST_COLS], F32, kind="ExternalInput")
    idxs = nc.dram_tensor("idxs", [128, IDX_COLS], I16, kind="ExternalInput")
    lt2 = nc.dram_tensor("lt2", [NROI_CORE, 128, L2['NCH'], NBIN], F32, kind="ExternalInput")
    lt3 = nc.dram_tensor("lt3", [NROI_CORE, 128, NBIN], F32, kind="ExternalInput")
    out = nc.dram_tensor("out", [NROI_CORE, C, NBIN], F32, kind="ExternalOutput")

    # overlapping 3-px elem view of f0: stride 2px, width 3px
    f0_view = AP(f0, 0, [[2 * C, F0_ROWS // 2 - 1], [1, 3 * C]])
    gather_srcs = [f0_view, f1[:], f2[:], f3[:]]
    ELEM = [3 * C, C, C, C]
    STEP = [2 * C, C, C, C]
    ICOLS = [IC0, IC1, IC2, IC3]
    IOFF = [IDX0_OFF, IDX1_OFF, IDX2_OFF, IDX3_OFF]

    with tile.TileContext(nc) as tc:
        with (
            tc.tile_pool(name="const", bufs=1) as constp,
            tc.tile_pool(name="g0p", bufs=2) as g0p,
            tc.tile_pool(name="g1p", bufs=2) as g1p,
            tc.tile_pool(name="g2p", bufs=2) as g2p,
            tc.tile_pool(name="g3p", bufs=2) as g3p,
            tc.tile_pool(name="ltp", bufs=3) as ltp,
            tc.tile_pool(name="wp", bufs=6) as wp,
            tc.tile_pool(name="accp", bufs=4, space="PSUM") as accp,
            tc.tile_pool(name="ptp", bufs=2, space="PSUM") as ptp,
            tc.tile_pool(name="evp", bufs=3) as evp,
            tc.tile_pool(name="otp", bufs=3) as otp,
        ):
            cst_t = constp.tile([128, CST_COLS], F32)
            nc.sync.dma_start(cst_t[:], cst[:])
            idx_t = constp.tile([128, IDX_COLS], I16)
            nc.sync.dma_start(idx_t[:], idxs[:])

            gpools = [g0p, g1p, g2p, g3p]
            for grp in range(NGRP):
                gts = []
                for l, lv in enumerate(LEVELS):
                    nidx = GRP * lv['NJ']
                    gt = gpools[l].tile([128, GRP * lv['NCH'], ELEM[l]], F32,
                                        tag=f"g{l}")
                    io = IOFF[l] + grp * ICOLS[l]
                    if nidx <= 1024:
                        nc.gpsimd.dma_gather(
                            gt[:], gather_srcs[l], idx_t[:, io:io + ICOLS[l]],
                            nidx, nidx, ELEM[l], elem_step=STEP[l])
                    else:
                        # SWDGE ring cap: split into one call per ROI
                        hc = ICOLS[l] // GRP
                        for r2 in range(GRP):
                            nc.gpsimd.dma_gather(
                                gt[:, r2 * lv['NCH']:(r2 + 1) * lv['NCH'], :],
                                gather_srcs[l],
                                idx_t[:, io + r2 * hc:io + (r2 + 1) * hc],
                                lv['NJ'], lv['NJ'], ELEM[l], elem_step=STEP[l])
                    gts.append(gt)

                for r2 in range(GRP):
                    roi = grp * GRP + r2
                    lt2_t = ltp.tile([128, L2['NCH'], NBIN], F32, tag="lt2")
                    nc.sync.dma_start(lt2_t[:], lt2[roi])
                    lt3_t = ltp.tile([128, NBIN], F32, tag="lt3")
                    nc.sync.dma_start(lt3_t[:], lt3[roi])

                    acc = accp.tile([NBIN, C], F32)
                    n_mm = 12 + 7 + 3 + 1
                    mi = 0
                    # L0: 4 chunks x 3 slots
                    for c in range(L0['NCH']):
                        for s in range(3):
                            w = wp.tile([128, NBIN], F32, tag="w")
                            colw = WCOL0_OFF + roi * 12 + c * 3 + s
                            nc.vector.tensor_scalar_mul(
                                w[:],
                                cst_t[:, PAT0_OFF + c * NBIN:PAT0_OFF + (c + 1) * NBIN],
                                cst_t[:, colw:colw + 1])
                            nc.tensor.matmul(
                                acc[:], w[:],
                                gts[0][:, r2 * L0['NCH'] + c, s * C:(s + 1) * C],
                                start=(mi == 0), stop=(mi == n_mm - 1))
                            mi += 1
                    # L1: 7 chunks
                    for c in range(L1['NCH']):
                        w = wp.tile([128, NBIN], F32, tag="w")
                        colw = WCOL1_OFF + roi * 7 + c
                        nc.vector.tensor_scalar_mul(
                            w[:],
                            cst_t[:, PAT1_OFF + c * NBIN:PAT1_OFF + (c + 1) * NBIN],
                            cst_t[:, colw:colw + 1])
                        nc.tensor.matmul(
                            acc[:], w[:], gts[1][:, r2 * L1['NCH'] + c, :],
                            start=(mi == 0), stop=(mi == n_mm - 1))
                        mi += 1
                    # L2: 3 chunks, host-baked lhsT
                    for c in range(L2['NCH']):
                        nc.tensor.matmul(
                            acc[:], lt2_t[:, c, :], gts[2][:, r2 * L2['NCH'] + c, :],
                            start=(mi == 0), stop=(mi == n_mm - 1))
                        mi += 1
                    # L3: 1 chunk
                    nc.tensor.matmul(
                        acc[:], lt3_t[:], gts[3][:, r2, :],
                        start=(mi == 0), stop=(mi == n_mm - 1))
                    mi += 1

                    ev = evp.tile([NBIN, C], F32, tag="ev")
                    nc.scalar.copy(ev[:], acc[:])
                    pt = ptp.tile([128, 2, NBIN], F32, tag="pt")
                    for h in range(2):
                        nc.tensor.transpose(
                            pt[:, h, :], ev[:, h * 128:(h + 1) * 128],
                            cst_t[:NBIN, ID_OFF:ID_OFF + NBIN])
                    ot = otp.tile([128, 2, NBIN], F32, tag="ot")
                    nc.vector.tensor_copy(ot[:], pt[:])
                    # out[roi] is [256, 49]; view as [h, p, m] -> dst [p, h, m]
                    dst = out[roi].rearrange("(h p) m -> p h m", h=2)
                    nc.sync.dma_start(dst, ot[:])
    nc.finalize()
    return nc


def kernel(x0, x1, x2, x3, boxes):
    from concourse.bass_utils import run_bass_kernel_spmd
    in_maps = _host_prepare(x0, x1, x2, x3, boxes)
    if 'nc' not in _MODULE_CACHE:
        _MODULE_CACHE['nc'] = _build_module()
    nc = _MODULE_CACHE['nc']
    res = run_bass_kernel_spmd(nc, in_maps, list(range(8)))
    _MODULE_CACHE['last_result'] = res
    outs = [res.results[k]["out"] for k in range(8)]
    full = np.concatenate(outs, axis=0)           # [1024, 256, 49]
    return full.reshape(1024, C, POOLED, POOLED).astype(np.float32)



# revision 4
# speedup vs baseline: 1.4278x; 1.4278x over previous
"""Multi-level ROI Align (FPN pooler, 4 levels summed) on 8 Trainium2 cores.

Strategy: shard ROIs across cores (core k: batch k//4, 128 ROIs). All gather
indices and bilinear weights are computed on host from `boxes`; the device
kernel does the heavy lifting: HBM pixel gathers (dma_gather) + weighted
scatter-reduction into 7x7 bins via PSUM-accumulating matmuls.

Per ROI, per level:
  out[bin, c] = sum_j W[j, bin] * G[j, c]
where G rows are gathered pixel vectors (C=256) and W is sparse (built on
device as fixed_pattern * per-partition scalar for L0/L1, host-baked dense
for the region-gathered L2/L3).

L0 uses 3-pixel elements addressed at even-pixel granularity (idx = flat//2)
to fit the int16 index range (200*200 = 40000 > 32767).
"""
import sys
import numpy as np

sys.path.insert(0, '/opt/trn_rl_repo')

POOLED = 7
SAMP = 2
NBIN = 49
C = 256
IMG = 800.0

# per level: H, W, scale, mode
#   mode 'tri': 3-px elems, idx=flat//2, NJ j's with 3 weight slots
#   mode 'px' : 1-px elems, corner gathers
#   mode 'reg': 1-px elems, bounding-region pixels, host-baked lhsT
L0 = dict(H=200, W=200, scale=0.25, mode='tri', NJ=512, REAL=392, NCH=4)
L1 = dict(H=100, W=100, scale=0.125, mode='px', NJ=896, REAL=784, NCH=7)
L2 = dict(H=50, W=50, scale=0.0625, mode='reg', NJ=384, REAL=324, NCH=3, WREG=18)
L3 = dict(H=25, W=25, scale=0.03125, mode='reg', NJ=128, REAL=100, NCH=1, WREG=10)
LEVELS = [L0, L1, L2, L3]

NROI_CORE = 128     # ROIs per core
NGRP = 64           # groups of 2 ROIs
GRP = 2

# padded flat pixel counts of the feature buffers
F0_ROWS = 40004     # covers 3-px elem overrun
F1_ROWS = 10000
F2_ROWS = 3400      # covers region overrun (y,x up to 66)
F3_ROWS = 900       # covers region overrun (y,x up to 33)

# const fp32 column layout (per partition)
PAT0_OFF = 0                       # [4, 49]
PAT1_OFF = PAT0_OFF + 4 * NBIN     # [7, 49]
WCOL0_OFF = PAT1_OFF + 7 * NBIN    # [128 roi * 12]
WCOL1_OFF = WCOL0_OFF + NROI_CORE * 12   # [128 roi * 7]
ID_OFF = WCOL1_OFF + NROI_CORE * 7       # [49]
CST_COLS = ID_OFF + NBIN

# idx int16 column layout (per partition), per 2-ROI group
IC0, IC1, IC2, IC3 = 64, 112, 48, 16     # cols per group per level
IDX0_OFF = 0
IDX1_OFF = IDX0_OFF + NGRP * IC0
IDX2_OFF = IDX1_OFF + NGRP * IC1
IDX3_OFF = IDX2_OFF + NGRP * IC2
IDX_COLS = IDX3_OFF + NGRP * IC3

_MODULE_CACHE = {}


def _sample_meta(boxes_b, H, W, scale):
    """Per-ROI sample geometry in fp32, matching reference op order.
    boxes_b: [N, 4] fp32. Returns dict of [N,7,2] arrays."""
    f = np.float32
    b = boxes_b.astype(np.float32)
    x1 = b[:, 0] * f(scale)
    y1 = b[:, 1] * f(scale)
    x2 = b[:, 2] * f(scale)
    y2 = b[:, 3] * f(scale)
    rw = np.maximum(x2 - x1, f(1.0))
    rh = np.maximum(y2 - y1, f(1.0))
    bw = rw / f(POOLED)
    bh = rh / f(POOLED)
    g = (np.arange(POOLED, dtype=np.float32)[:, None]
         + (np.arange(SAMP, dtype=np.float32)[None, :] + f(0.5)) / f(SAMP))
    y = y1[:, None, None] + g[None] * bh[:, None, None]   # [N,7,2]
    x = x1[:, None, None] + g[None] * bw[:, None, None]
    masky = ((y >= f(-1.0)) & (y <= f(H))).astype(np.float32)
    maskx = ((x >= f(-1.0)) & (x <= f(W))).astype(np.float32)
    yc = np.clip(y, f(0.0), f(H - 1))
    xc = np.clip(x, f(0.0), f(W - 1))
    yl = np.floor(yc).astype(np.int64)
    xl = np.floor(xc).astype(np.int64)
    yh = np.minimum(yl + 1, H - 1)
    xh = np.minimum(xl + 1, W - 1)
    ly = (yc - yl.astype(np.float32)).astype(np.float32)
    lx = (xc - xl.astype(np.float32)).astype(np.float32)
    hy = (f(1.0) - ly).astype(np.float32)
    hx = (f(1.0) - lx).astype(np.float32)
    return dict(yl=yl, yh=yh, xl=xl, xh=xh, ly=ly, lx=lx, hy=hy, hx=hx,
                masky=masky, maskx=maskx, x=x, y=y)


def _build_tri(meta, lv):
    """L0: j = (row_sel, py, sy, px, sx) -> 392 3-px elems, 3 slot weights.
    Returns idx [N, NJ] int64, w [N, NJ, 3] fp32."""
    N = meta['yl'].shape[0]
    W = lv['W']
    NJ, REAL = lv['NJ'], lv['REAL']
    rows = np.stack([meta['yl'], meta['yh']], axis=1)          # [N,2,7,2] (rs)
    wys = np.stack([meta['hy'], meta['ly']], axis=1)           # [N,2,7,2]
    m = (meta['masky'][:, :, :, None, None] * meta['maskx'][:, None, None, :, :])  # [N,7,2,7,2]
    # broadcast to [N, rs, py, sy, px, sx]
    row = np.broadcast_to(rows[:, :, :, :, None, None], (N, 2, 7, 2, 7, 2))
    wy = np.broadcast_to(wys[:, :, :, :, None, None], (N, 2, 7, 2, 7, 2)).astype(np.float32)
    xl = np.broadcast_to(meta['xl'][:, None, None, None, :, :], (N, 2, 7, 2, 7, 2))
    hx = np.broadcast_to(meta['hx'][:, None, None, None, :, :], (N, 2, 7, 2, 7, 2)).astype(np.float32)
    lx = np.broadcast_to(meta['lx'][:, None, None, None, :, :], (N, 2, 7, 2, 7, 2)).astype(np.float32)
    mm = np.broadcast_to(m[:, None], (N, 2, 7, 2, 7, 2)).astype(np.float32)
    flat = row * W + xl
    idx = (flat >> 1).reshape(N, REAL)
    r = (flat & 1).astype(np.float32).reshape(N, REAL)
    wl = (wy * hx * mm * np.float32(0.25)).reshape(N, REAL)
    wh = (wy * lx * mm * np.float32(0.25)).reshape(N, REAL)
    w = np.zeros((N, NJ, 3), np.float32)
    w[:, :REAL, 0] = wl * (1 - r)
    w[:, :REAL, 1] = wl * r + wh * (1 - r)
    w[:, :REAL, 2] = wh * r
    idx_full = np.zeros((N, NJ), np.int64)
    idx_full[:, :REAL] = idx
    return idx_full, w


def _build_px(meta, lv):
    """L1: j = (row_sel, col_sel, py, sy, px, sx) -> 784 1-px corner gathers.
    Returns idx [N, NJ] int64, w [N, NJ] fp32."""
    N = meta['yl'].shape[0]
    W = lv['W']
    NJ, REAL = lv['NJ'], lv['REAL']
    rows = np.stack([meta['yl'], meta['yh']], axis=1)   # [N,2(rs),7,2]
    wys = np.stack([meta['hy'], meta['ly']], axis=1)
    cols = np.stack([meta['xl'], meta['xh']], axis=1)   # [N,2(cs),7,2]
    wxs = np.stack([meta['hx'], meta['lx']], axis=1)
    m = (meta['masky'][:, :, :, None, None] * meta['maskx'][:, None, None, :, :])
    row = np.broadcast_to(rows[:, :, None, :, :, None, None], (N, 2, 2, 7, 2, 7, 2))
    wy = np.broadcast_to(wys[:, :, None, :, :, None, None], (N, 2, 2, 7, 2, 7, 2)).astype(np.float32)
    col = np.broadcast_to(cols[:, None, :, None, None, :, :], (N, 2, 2, 7, 2, 7, 2))
    wx = np.broadcast_to(wxs[:, None, :, None, None, :, :], (N, 2, 2, 7, 2, 7, 2)).astype(np.float32)
    mm = np.broadcast_to(m[:, None, None], (N, 2, 2, 7, 2, 7, 2)).astype(np.float32)
    idx = (row * W + col).reshape(N, REAL)
    w = (wy * wx * mm * np.float32(0.25)).reshape(N, REAL)
    idx_full = np.zeros((N, NJ), np.int64)
    w_full = np.zeros((N, NJ), np.float32)
    idx_full[:, :REAL] = idx
    w_full[:, :REAL] = w
    return idx_full, w_full


def _build_reg(meta, lv):
    """L2/L3: bounding-region pixels + separable host-baked weights.
    Returns idx [N, NJ] int64, lhsT [N, NJ, 49] fp32."""
    N = meta['yl'].shape[0]
    H, W, WREG = lv['H'], lv['W'], lv['WREG']
    NJ, REAL = lv['NJ'], lv['REAL']
    f = np.float32
    y_base = np.floor(np.clip(meta['y'].reshape(N, -1).min(1), 0.0, H - 1)).astype(np.int64)
    x_base = np.floor(np.clip(meta['x'].reshape(N, -1).min(1), 0.0, W - 1)).astype(np.int64)
    # WY [N, WREG, 7], WX [N, WREG, 7]
    WY = np.zeros((N, WREG, POOLED), np.float32)
    WX = np.zeros((N, WREG, POOLED), np.float32)
    ridx = np.arange(N)[:, None, None]
    pidx = np.broadcast_to(np.arange(POOLED)[None, :, None], (N, POOLED, SAMP))
    np.add.at(WY, (ridx, meta['yl'] - y_base[:, None, None], pidx),
              (f(0.5) * meta['hy'] * meta['masky']).astype(np.float32))
    np.add.at(WY, (ridx, meta['yh'] - y_base[:, None, None], pidx),
              (f(0.5) * meta['ly'] * meta['masky']).astype(np.float32))
    np.add.at(WX, (ridx, meta['xl'] - x_base[:, None, None], pidx),
              (f(0.5) * meta['hx'] * meta['maskx']).astype(np.float32))
    np.add.at(WX, (ridx, meta['xh'] - x_base[:, None, None], pidx),
              (f(0.5) * meta['lx'] * meta['maskx']).astype(np.float32))
    lhsT = np.einsum('nap,nbq->nabpq', WY, WX).reshape(N, REAL, NBIN)
    dy = np.arange(WREG)
    idx = ((y_base[:, None, None] + dy[None, :, None]) * W
           + x_base[:, None, None] + dy[None, None, :]).reshape(N, REAL)
    idx_full = np.zeros((N, NJ), np.int64)
    lhsT_full = np.zeros((N, NJ, NBIN), np.float32)
    idx_full[:, :REAL] = idx
    lhsT_full[:, :REAL] = lhsT
    return idx_full, lhsT_full


def _pack_idx(jlists):
    """Pack concatenated per-group idx list [NJ_total] -> [128, NJ_total//16]
    int16 wrapped in 16 partitions, replicated 8x."""
    jl = np.asarray(jlists)
    n = jl.shape[-1]
    arr = jl.reshape(*jl.shape[:-1], n // 16, 16)   # [..., col, p]
    arr = np.swapaxes(arr, -1, -2)                  # [..., p(16), col]
    arr = np.broadcast_to(arr[..., None, :, :],
                          (*jl.shape[:-1], 8, 16, n // 16))
    return arr.reshape(*jl.shape[:-1], 128, n // 16).astype(np.int16)


def _bin_pattern(mode, NCH, REAL):
    """Fixed j->bin one-hot pattern [128, NCH, 49] for 'tri'/'px' j order."""
    NJ = NCH * 128
    j = np.arange(NJ)
    if mode == 'tri':
        # j = ((((rs*7+py)*2+sy)*7+px)*2+sx)
        px = (j // 2) % 7
        py = (j // (2 * 7 * 2)) % 7
    else:
        # j = (((((rs*2+cs)*7+py)*2+sy)*7+px)*2+sx)
        px = (j // 2) % 7
        py = (j // (2 * 7 * 2)) % 7
    bins = py * 7 + px
    pat = np.zeros((NJ, NBIN), np.float32)
    valid = j < REAL
    pat[np.arange(NJ)[valid], bins[valid]] = 1.0
    return pat.reshape(NCH, 128, NBIN).transpose(1, 0, 2)   # [128, NCH, 49]


def _host_prepare(x0, x1, x2, x3, boxes):
    """Build all per-core input tensors. Returns list of 8 dicts."""
    B = boxes.shape[0]
    feats = []
    for arr, lv, rows in ((x0, L0, F0_ROWS), (x1, L1, F1_ROWS),
                          (x2, L2, F2_ROWS), (x3, L3, F3_ROWS)):
        f = np.zeros((B, rows, C), np.float32)
        hw = lv['H'] * lv['W']
        f[:, :hw] = np.ascontiguousarray(
            np.transpose(np.asarray(arr, np.float32), (0, 2, 3, 1))).reshape(B, hw, C)
        feats.append(f)

    per_batch = []
    for b in range(B):
        bb = np.asarray(boxes[b], np.float32)
        m0 = _sample_meta(bb, L0['H'], L0['W'], L0['scale'])
        m1 = _sample_meta(bb, L1['H'], L1['W'], L1['scale'])
        m2 = _sample_meta(bb, L2['H'], L2['W'], L2['scale'])
        m3 = _sample_meta(bb, L3['H'], L3['W'], L3['scale'])
        idx0, w0 = _build_tri(m0, L0)
        idx1, w1 = _build_px(m1, L1)
        idx2, lt2 = _build_reg(m2, L2)
        idx3, lt3 = _build_reg(m3, L3)
        per_batch.append((idx0, w0, idx1, w1, idx2, lt2, idx3, lt3))

    pat0 = _bin_pattern('tri', L0['NCH'], L0['REAL'])
    pat1 = _bin_pattern('px', L1['NCH'], L1['REAL'])

    in_maps = []
    for k in range(8):
        b = k // 4
        s = (k % 4) * NROI_CORE
        idx0, w0, idx1, w1, idx2, lt2, idx3, lt3 = per_batch[b]
        sl = slice(s, s + NROI_CORE)

        cst = np.zeros((128, CST_COLS), np.float32)
        cst[:, PAT0_OFF:PAT0_OFF + 4 * NBIN] = pat0.reshape(128, -1)
        cst[:, PAT1_OFF:PAT1_OFF + 7 * NBIN] = pat1.reshape(128, -1)
        # wcol0 [128, roi*12]: col roi*12 + c*3 + slot = w0[roi, c*128+p, slot]
        wc0 = w0[sl].reshape(NROI_CORE, L0['NCH'], 128, 3)   # [roi,c,p,s]
        cst[:, WCOL0_OFF:WCOL0_OFF + NROI_CORE * 12] = (
            wc0.transpose(2, 0, 1, 3).reshape(128, -1))
        wc1 = w1[sl].reshape(NROI_CORE, L1['NCH'], 128)      # [roi,c,p]
        cst[:, WCOL1_OFF:WCOL1_OFF + NROI_CORE * 7] = (
            wc1.transpose(2, 0, 1).reshape(128, -1))
        cst[:NBIN, ID_OFF:ID_OFF + NBIN] = np.eye(NBIN, dtype=np.float32)

        idxs = np.zeros((128, IDX_COLS), np.int16)
        idxs[:, IDX0_OFF:IDX0_OFF + NGRP * IC0] = _pack_idx(
            idx0[sl].reshape(NGRP, GRP * L0['NJ'])).transpose(1, 0, 2).reshape(128, -1)
        idxs[:, IDX1_OFF:IDX1_OFF + NGRP * IC1] = _pack_idx(
            idx1[sl].reshape(NGRP, GRP * L1['NJ'])).transpose(1, 0, 2).reshape(128, -1)
        idxs[:, IDX2_OFF:IDX2_OFF + NGRP * IC2] = _pack_idx(
            idx2[sl].reshape(NGRP, GRP * L2['NJ'])).transpose(1, 0, 2).reshape(128, -1)
        idxs[:, IDX3_OFF:IDX3_OFF + NGRP * IC3] = _pack_idx(
            idx3[sl].reshape(NGRP, GRP * L3['NJ'])).transpose(1, 0, 2).reshape(128, -1)

        # lhsT k-major: lt2 [roi, NJ(=3*128), 49] -> [roi, 128, 3, 49]
        lt2k = np.ascontiguousarray(
            lt2[sl].reshape(NROI_CORE, L2['NCH'], 128, NBIN).transpose(0, 2, 1, 3))
        lt3k = np.ascontiguousarray(lt3[sl].reshape(NROI_CORE, 128, NBIN))

        in_maps.append({
            "f0": feats[0][b], "f1": feats[1][b],
            "f2": feats[2][b], "f3": feats[3][b],
            "cst": cst, "idxs": idxs, "lt2": lt2k, "lt3": lt3k,
        })
    return in_maps


def _build_module():
    from concourse import bacc, tile
    from concourse.bass import mybir
    import concourse.bass as bass_mod

    F32 = mybir.dt.float32
    I16 = mybir.dt.int16
    AP = bass_mod.AP

    nc = bacc.Bacc(None, target_bir_lowering=False, num_swdge_queues=4)
    f0 = nc.dram_tensor("f0", [F0_ROWS, C], F32, kind="ExternalInput")
    f1 = nc.dram_tensor("f1", [F1_ROWS, C], F32, kind="ExternalInput")
    f2 = nc.dram_tensor("f2", [F2_ROWS, C], F32, kind="ExternalInput")
    f3 = nc.dram_tensor("f3", [F3_ROWS, C], F32, kind="ExternalInput")
    cst = nc.dram_tensor("cst", [128, CST_COLS], F32, kind="ExternalInput")
    idxs = nc.dram_tensor("idxs", [128, IDX_COLS], I16, kind="ExternalInput")
    lt2 = nc.dram_tensor("lt2", [NROI_CORE, 128, L2['NCH'], NBIN], F32, kind="ExternalInput")
    lt3 = nc.dram_tensor("lt3", [NROI_CORE, 128, NBIN], F32, kind="ExternalInput")
    out = nc.dram_tensor("out", [NROI_CORE, C, NBIN], F32, kind="ExternalOutput")

    # overlapping 3-px elem view of f0: stride 2px, width 3px
    f0_view = AP(f0, 0, [[2 * C, F0_ROWS // 2 - 1], [1, 3 * C]])
    gather_srcs = [f0_view, f1[:], f2[:], f3[:]]
    ELEM = [3 * C, C, C, C]
    STEP = [2 * C, C, C, C]
    ICOLS = [IC0, IC1, IC2, IC3]
    IOFF = [IDX0_OFF, IDX1_OFF, IDX2_OFF, IDX3_OFF]

    with tile.TileContext(nc) as tc:
        with (
            tc.tile_pool(name="const", bufs=1) as constp,
            tc.tile_pool(name="g0p", bufs=2) as g0p,
            tc.tile_pool(name="g1p", bufs=2) as g1p,
            tc.tile_pool(name="g2p", bufs=2) as g2p,
            tc.tile_pool(name="g3p", bufs=2) as g3p,
            tc.tile_pool(name="ltp", bufs=3) as ltp,
            tc.tile_pool(name="wp", bufs=6) as wp,
            tc.tile_pool(name="accp", bufs=4, space="PSUM") as accp,
            tc.tile_pool(name="ptp", bufs=2, space="PSUM") as ptp,
            tc.tile_pool(name="evp", bufs=3) as evp,
            tc.tile_pool(name="otp", bufs=3) as otp,
        ):
            cst_t = constp.tile([128, CST_COLS], F32)
            nc.sync.dma_start(cst_t[:], cst[:])
            idx_t = constp.tile([128, IDX_COLS], I16)
            nc.sync.dma_start(idx_t[:], idxs[:])

            gpools = [g0p, g1p, g2p, g3p]
            for grp in range(NGRP):
                gts = []
                for l, lv in enumerate(LEVELS):
                    nidx = GRP * lv['NJ']
                    gt = gpools[l].tile([128, GRP * lv['NCH'], ELEM[l]], F32,
                                        tag=f"g{l}")
                    io = IOFF[l] + grp * ICOLS[l]
                    if nidx <= 1024:
                        nc.gpsimd.dma_gather(
                            gt[:], gather_srcs[l], idx_t[:, io:io + ICOLS[l]],
                            nidx, nidx, ELEM[l], elem_step=STEP[l],
                            queue_num=l % 4)
                    else:
                        # SWDGE ring cap: split into one call per ROI
                        hc = ICOLS[l] // GRP
                        for r2 in range(GRP):
                            nc.gpsimd.dma_gather(
                                gt[:, r2 * lv['NCH']:(r2 + 1) * lv['NCH'], :],
                                gather_srcs[l],
                                idx_t[:, io + r2 * hc:io + (r2 + 1) * hc],
                                lv['NJ'], lv['NJ'], ELEM[l], elem_step=STEP[l],
                                queue_num=(l + r2) % 4)
                    gts.append(gt)

                for r2 in range(GRP):
                    roi = grp * GRP + r2
                    lt2_t = ltp.tile([128, L2['NCH'], NBIN], F32, tag="lt2")
                    nc.sync.dma_start(lt2_t[:], lt2[roi])
                    lt3_t = ltp.tile([128, NBIN], F32, tag="lt3")
                    nc.sync.dma_start(lt3_t[:], lt3[roi])

                    acc = accp.tile([NBIN, C], F32)
                    n_mm = 12 + 7 + 3 + 1
                    mi = 0
                    # L0: 4 chunks x 3 slots
                    for c in range(L0['NCH']):
                        for s in range(3):
                            w = wp.tile([128, NBIN], F32, tag="w")
                            colw = WCOL0_OFF + roi * 12 + c * 3 + s
                            nc.vector.tensor_scalar_mul(
                                w[:],
                                cst_t[:, PAT0_OFF + c * NBIN:PAT0_OFF + (c + 1) * NBIN],
                                cst_t[:, colw:colw + 1])
                            nc.tensor.matmul(
                                acc[:], w[:],
                                gts[0][:, r2 * L0['NCH'] + c, s * C:(s + 1) * C],
                                start=(mi == 0), stop=(mi == n_mm - 1))
                            mi += 1
                    # L1: 7 chunks
                    for c in range(L1['NCH']):
                        w = wp.tile([128, NBIN], F32, tag="w")
                        colw = WCOL1_OFF + roi * 7 + c
                        nc.vector.tensor_scalar_mul(
                            w[:],
                            cst_t[:, PAT1_OFF + c * NBIN:PAT1_OFF + (c + 1) * NBIN],
                            cst_t[:, colw:colw + 1])
                        nc.tensor.matmul(
                            acc[:], w[:], gts[1][:, r2 * L1['NCH'] + c, :],
                            start=(mi == 0), stop=(mi == n_mm - 1))
                        mi += 1
                    # L2: 3 chunks, host-baked lhsT
                    for c in range(L2['NCH']):
                        nc.tensor.matmul(
                            acc[:], lt2_t[:, c, :], gts[2][:, r2 * L2['NCH'] + c, :],
                            start=(mi == 0), stop=(mi == n_mm - 1))
                        mi += 1
                    # L3: 1 chunk
                    nc.tensor.matmul(
                        acc[:], lt3_t[:], gts[3][:, r2, :],
                        start=(mi == 0), stop=(mi == n_mm - 1))
                    mi += 1

                    ev = evp.tile([NBIN, C], F32, tag="ev")
                    nc.scalar.copy(ev[:], acc[:])
                    pt = ptp.tile([128, 2, NBIN], F32, tag="pt")
                    for h in range(2):
                        nc.tensor.transpose(
                            pt[:, h, :], ev[:, h * 128:(h + 1) * 128],
                            cst_t[:NBIN, ID_OFF:ID_OFF + NBIN])
                    ot = otp.tile([128, 2, NBIN], F32, tag="ot")
                    nc.vector.tensor_copy(ot[:], pt[:])
                    # out[roi] is [256, 49]; view as [h, p, m] -> dst [p, h, m]
                    dst = out[roi].rearrange("(h p) m -> p h m", h=2)
                    nc.sync.dma_start(dst, ot[:])
    nc.finalize()
    return nc


def kernel(x0, x1, x2, x3, boxes):
    from concourse.bass_utils import run_bass_kernel_spmd
    in_maps = _host_prepare(x0, x1, x2, x3, boxes)
    if 'nc' not in _MODULE_CACHE:
        _MODULE_CACHE['nc'] = _build_module()
    nc = _MODULE_CACHE['nc']
    res = run_bass_kernel_spmd(nc, in_maps, list(range(8)))
    _MODULE_CACHE['last_result'] = res
    outs = [res.results[k]["out"] for k in range(8)]
    full = np.concatenate(outs, axis=0)           # [1024, 256, 49]
    return full.reshape(1024, C, POOLED, POOLED).astype(np.float32)



# revision 10
# speedup vs baseline: 2.6492x; 1.8554x over previous
"""Multi-level ROI Align (FPN pooler, 4 levels summed) on 8 Trainium2 cores.

v3: descriptor-minimized fp16 design. Shard ROIs across cores (core k:
batch k//4, 128 ROIs as 64 pairs). Host computes all gather indices and
bilinear weights from `boxes`; device does HBM strip-gathers (dma_gather,
one descriptor per multi-pixel strip) + fp16 matmuls accumulating both
ROIs of a pair into one PSUM tile [98, 256] (bins 0-48 = ROI a, 49-97 =
ROI b), evicted straight to DRAM as [pair, 98, 256] fp32; host transposes.

Per pair-of-ROIs:
  L0 (200x200, s=.25):  2x196 strips of 8px (even-aligned, idx=flat>>1),
      one strip per (rowsel, ysample, xbin); rank-1 weights
      (fixed one-hot bin pattern x per-strip scalar, built by one DVE op)
      -> 4 chunks x 8 slots = 32 matmuls
  L1 (100x100, s=.125): 2x112 strips of 9px, one per (rowsel, ysample,
      xbin-pair); host-baked dense lhsT -> 2 chunks x 9 slots = 18 mm
  L2 (50x50, s=.0625):  2x56 strips of 10px, one per (rowsel, ysample,
      xbin-quad); dense lhsT -> 1 chunk x 10 slots = 10 mm
  L3 (25x25, s=.03125): 2x100 px of the 10x10 bounding region; dense
      separable lhsT -> 2 chunks x 1 slot = 2 mm
"""
import sys
import numpy as np

sys.path.insert(0, '/opt/trn_rl_repo')

POOLED = 7
SAMP = 2
NBIN = 49
C = 256
IMG = 800.0

NROI_CORE = 128
NPAIR = 64

# per-level geometry
L0 = dict(H=200, W=200, scale=0.25)
L1 = dict(H=100, W=100, scale=0.125)
L2 = dict(H=50, W=50, scale=0.0625)
L3 = dict(H=25, W=25, scale=0.03125)

W0, W1, W2, W3 = 8, 9, 10, 10          # strip widths (px)
NS0, NS1, NS2 = 196, 112, 56           # strips per ROI
NR3 = 100                              # region px per ROI
N0, N1, N2, N3 = 400, 224, 112, 208    # padded pair nidx (%16, incl -1 tail)
CH0, CH1, CH2, CH3 = 4, 2, 1, 2        # chunks per pair
NT1, NT2, NT3 = CH1 * W1, W2, CH3      # dense lhsT tiles per level
NTD = NT1 + NT2 + NT3                  # 18 + 10 + 2 = 30
NBIN2 = 2 * NBIN                       # 98

F0_ROWS = 40004
F1_ROWS = 10000
F2_ROWS = 2500
F3_ROWS = 864

# cst fp16 column layout: L0 pattern [4ch, 98] then L0 scalars
PAT0_COLS = CH0 * NBIN2                        # 392
SCAL0_PER_PAIR = CH0 * W0                      # 32
CST_COLS = PAT0_COLS + NPAIR * SCAL0_PER_PAIR  # 392 + 2048

# idx int16 layout per pair: [L0 400][L1 224][L2 112][L3 208] / 16
PAIR_ICOLS = (N0 + N1 + N2 + N3) // 16         # 59
IDX_COLS = NPAIR * PAIR_ICOLS

_MODULE_CACHE = {}


def _sample_meta(boxes_b, H, W, scale):
    """Per-ROI sample geometry in fp32, matching reference op order.
    boxes_b: [N, 4] fp32. Returns dict of [N,7,2] arrays."""
    f = np.float32
    b = boxes_b.astype(np.float32)
    x1 = b[:, 0] * f(scale)
    y1 = b[:, 1] * f(scale)
    x2 = b[:, 2] * f(scale)
    y2 = b[:, 3] * f(scale)
    rw = np.maximum(x2 - x1, f(1.0))
    rh = np.maximum(y2 - y1, f(1.0))
    bw = rw / f(POOLED)
    bh = rh / f(POOLED)
    g = (np.arange(POOLED, dtype=np.float32)[:, None]
         + (np.arange(SAMP, dtype=np.float32)[None, :] + f(0.5)) / f(SAMP))
    y = y1[:, None, None] + g[None] * bh[:, None, None]   # [N,7,2]
    x = x1[:, None, None] + g[None] * bw[:, None, None]
    masky = ((y >= f(-1.0)) & (y <= f(H))).astype(np.float32)
    maskx = ((x >= f(-1.0)) & (x <= f(W))).astype(np.float32)
    yc = np.clip(y, f(0.0), f(H - 1))
    xc = np.clip(x, f(0.0), f(W - 1))
    yl = np.floor(yc).astype(np.int64)
    xl = np.floor(xc).astype(np.int64)
    yh = np.minimum(yl + 1, H - 1)
    xh = np.minimum(xl + 1, W - 1)
    ly = (yc - yl.astype(np.float32)).astype(np.float32)
    lx = (xc - xl.astype(np.float32)).astype(np.float32)
    hy = (f(1.0) - ly).astype(np.float32)
    hx = (f(1.0) - lx).astype(np.float32)
    return dict(yl=yl, yh=yh, xl=xl, xh=xh, ly=ly, lx=lx, hy=hy, hx=hx,
                masky=masky, maskx=maskx, x=x, y=y)


def _strip_grid(meta, W, bins_per_strip, width, parity):
    """Build per-ROI strip indices and slot weights.

    Strips: (rowsel 2) x (ysample 14) x (xgroup ceil(7/b)).
    Returns idx [N, NS] int64 (pixel index of strip start, or start>>1 if
    parity), wslot [N, NS, width] fp32 (bilinear x-weights x y-weight x
    mask x 0.25), and bins [NS_xgroups arrays] for pattern building is
    implicit: each strip's samples' bins vary within the group -- handled
    by caller via per-sample info: also returns sample slot/bin arrays:
    contrib = (slotpos [N, NS, 7bins?..]) -- instead we return dense
    per-strip per-slot per-bin weights only when needed. For rank-1 (b=1)
    wslot is enough (all slots -> the strip's single bin).
    """
    N = meta['yl'].shape[0]
    f = np.float32
    ngrp = -(-POOLED // bins_per_strip)           # x-groups per row
    NS = 2 * 14 * ngrp
    # rows/yweights: [N, 2, 14]
    rows = np.stack([meta['yl'], meta['yh']], axis=1).reshape(N, 2, 14)
    wy = (np.stack([meta['hy'], meta['ly']], axis=1)
          * meta['masky'][:, None]).reshape(N, 2, 14).astype(np.float32)
    # x corners per sample: [N, 7, 2]
    xl, xh = meta['xl'], meta['xh']
    wxl = (meta['hx'] * meta['maskx']).astype(np.float32)
    wxh = (meta['lx'] * meta['maskx']).astype(np.float32)
    # group starts: min xl over samples in group -> [N, ngrp]
    xs = np.empty((N, ngrp), np.int64)
    for gi in range(ngrp):
        b0, b1 = gi * bins_per_strip, min((gi + 1) * bins_per_strip, POOLED)
        xs[:, gi] = xl[:, b0:b1, :].reshape(N, -1).min(axis=1)
    if parity:
        xs &= ~1
    xs = np.clip(xs, 0, W - width)
    # slot weights [N, ngrp, width] per (bin-in-group, sx): scatter
    wslot = np.zeros((N, ngrp, width, POOLED), np.float32)  # per-bin slots
    ridx = np.arange(N)[:, None, None]
    for gi in range(ngrp):
        b0, b1 = gi * bins_per_strip, min((gi + 1) * bins_per_strip, POOLED)
        for bx in range(b0, b1):
            for sx in range(SAMP):
                ol = xl[:, bx, sx] - xs[:, gi]
                oh = xh[:, bx, sx] - xs[:, gi]
                np.add.at(wslot, (np.arange(N), gi, ol, bx), wxl[:, bx, sx])
                np.add.at(wslot, (np.arange(N), gi, oh, bx), wxh[:, bx, sx])
    # combine with y: strips [N, 2, 14, ngrp]
    idx = (rows[:, :, :, None] * W + xs[:, None, None, :])
    if parity:
        idx >>= 1
    idx = idx.reshape(N, NS)
    # w [N, 2, 14, ngrp, width, 7] = wy * wslot * 0.25
    w = (wy[:, :, :, None, None, None]
         * wslot[:, None, None, :, :, :] * f(0.25))
    return idx, w.reshape(N, NS, width, POOLED)


def _build_l3_region(meta):
    """L3: 10x10 bounding region + separable dense weights.
    Returns idx [N, 100] int64, lhsT [N, 100, 49] fp32."""
    N = meta['yl'].shape[0]
    H, W, WREG = 25, 25, 10
    f = np.float32
    y_base = np.floor(np.clip(meta['y'].reshape(N, -1).min(1), 0.0, H - 1)).astype(np.int64)
    x_base = np.floor(np.clip(meta['x'].reshape(N, -1).min(1), 0.0, W - 1)).astype(np.int64)
    WY = np.zeros((N, WREG, POOLED), np.float32)
    WX = np.zeros((N, WREG, POOLED), np.float32)
    ridx = np.arange(N)[:, None, None]
    pidx = np.broadcast_to(np.arange(POOLED)[None, :, None], (N, POOLED, SAMP))
    np.add.at(WY, (ridx, meta['yl'] - y_base[:, None, None], pidx),
              (f(0.5) * meta['hy'] * meta['masky']).astype(np.float32))
    np.add.at(WY, (ridx, meta['yh'] - y_base[:, None, None], pidx),
              (f(0.5) * meta['ly'] * meta['masky']).astype(np.float32))
    np.add.at(WX, (ridx, meta['xl'] - x_base[:, None, None], pidx),
              (f(0.5) * meta['hx'] * meta['maskx']).astype(np.float32))
    np.add.at(WX, (ridx, meta['xh'] - x_base[:, None, None], pidx),
              (f(0.5) * meta['lx'] * meta['maskx']).astype(np.float32))
    lhsT = np.einsum('nap,nbq->nabpq', WY, WX).reshape(N, NR3, NBIN)
    dy = np.arange(WREG)
    idx = ((y_base[:, None, None] + dy[None, :, None]) * W
           + x_base[:, None, None] + dy[None, None, :]).reshape(N, NR3)
    return idx, lhsT


def _pack_idx(idx_flat):
    """[n] int -> [128, n//16] int16: wrap 16 partitions, replicate 8x."""
    n = idx_flat.shape[0]
    arr = idx_flat.reshape(n // 16, 16).T            # [16, cols]
    arr = np.broadcast_to(arr[None], (8, 16, n // 16)).reshape(128, n // 16)
    return arr.astype(np.int16)


def _l0_pattern():
    """Fixed one-hot [128, CH0, 98] fp16: J = c*128+p -> bin."""
    pat = np.zeros((CH0, 128, NBIN2), np.float16)
    for J in range(2 * NS0):
        half, j = J // NS0, J % NS0
        bx = j % POOLED
        t = (j // POOLED) % 14
        py = t // 2
        pat[J // 128, J % 128, py * 7 + bx + half * NBIN] = 1.0
    return pat.transpose(1, 0, 2)                    # [128, CH0, 98]


def _host_prepare(x0, x1, x2, x3, boxes):
    """Build all per-core input tensors. Returns list of 8 dicts."""
    B = boxes.shape[0]
    feats = []
    for arr, lv, rows in ((x0, L0, F0_ROWS), (x1, L1, F1_ROWS),
                          (x2, L2, F2_ROWS), (x3, L3, F3_ROWS)):
        f = np.zeros((B, rows, C), np.float16)
        hw = lv['H'] * lv['W']
        f[:, :hw] = np.ascontiguousarray(
            np.transpose(np.asarray(arr, np.float32), (0, 2, 3, 1))
        ).reshape(B, hw, C).astype(np.float16)
        feats.append(f)

    pat0 = _l0_pattern()

    per_batch = []
    for b in range(B):
        bb = np.asarray(boxes[b], np.float32)
        m0 = _sample_meta(bb, L0['H'], L0['W'], L0['scale'])
        m1 = _sample_meta(bb, L1['H'], L1['W'], L1['scale'])
        m2 = _sample_meta(bb, L2['H'], L2['W'], L2['scale'])
        m3 = _sample_meta(bb, L3['H'], L3['W'], L3['scale'])
        idx0, w0 = _strip_grid(m0, L0['W'], 1, W0, parity=True)   # [N,196],[N,196,8,7]
        idx1, w1 = _strip_grid(m1, L1['W'], 2, W1, parity=False)  # [N,112],[N,112,9,7]
        idx2, w2 = _strip_grid(m2, L2['W'], 4, W2, parity=False)  # [N,56],[N,56,10,7]
        idx3, lt3 = _build_l3_region(m3)                          # [N,100],[N,100,49]
        per_batch.append((idx0, w0, idx1, w1, idx2, w2, idx3, lt3))

    in_maps = []
    for k in range(8):
        b = k // 4
        s = (k % 4) * NROI_CORE
        idx0, w0, idx1, w1, idx2, w2, idx3, lt3 = per_batch[b]

        cst = np.zeros((128, CST_COLS), np.float16)
        cst[:, :PAT0_COLS] = pat0.reshape(128, -1)

        idxs = np.zeros((128, IDX_COLS), np.int16)
        wd = np.zeros((NPAIR, 128, NTD, NBIN2), np.float16)

        for p in range(NPAIR):
            ra, rb = s + 2 * p, s + 2 * p + 1
            # --- L0: scal table + idx
            j0 = np.full(N0, -1, np.int64)
            j0[:NS0] = idx0[ra]
            j0[NS0:2 * NS0] = idx0[rb]
            # scal0 [128part, CH0, W0]: J = c*128+pp -> strip J's slot weights
            sc = np.zeros((CH0 * 128, W0), np.float32)
            # per-strip scalar: rank-1 -> each strip has single bin; its
            # slot vector is w[...,:,bin_of_strip]
            wpair = np.concatenate([w0[ra], w0[rb]], axis=0)  # [392, 8, 7]
            bins_x = np.tile(np.arange(NS0) % POOLED, 2)
            sc[:2 * NS0] = wpair[np.arange(2 * NS0), :, bins_x]
            cst[:, PAT0_COLS + p * SCAL0_PER_PAIR:
                PAT0_COLS + (p + 1) * SCAL0_PER_PAIR] = (
                sc.reshape(CH0, 128, W0).transpose(1, 0, 2)
                .reshape(128, SCAL0_PER_PAIR).astype(np.float16))
            # --- L1: dense lhsT tiles 0..17
            j1 = np.empty(N1, np.int64)
            j1[:NS1] = idx1[ra]
            j1[NS1:] = idx1[rb]
            for half, r in ((0, ra), (1, rb)):
                for jj in range(NS1):
                    J = half * NS1 + jj
                    ch, pp = J // 128, J % 128
                    py = ((jj // 4) % 14) // 2
                    wv = w1[r, jj]          # [9, 7]
                    nz = np.nonzero(wv)
                    for o, bn in zip(*nz):
                        wd[p, pp, ch * W1 + o,
                           py * 7 + bn + half * NBIN] += wv[o, bn]
            # --- L2: dense lhsT tiles 18..27
            j2 = np.empty(N2, np.int64)
            j2[:NS2] = idx2[ra]
            j2[NS2:] = idx2[rb]
            for half, r in ((0, ra), (1, rb)):
                for jj in range(NS2):
                    J = half * NS2 + jj
                    pp = J % 128
                    py = ((jj // 2) % 14) // 2
                    wv = w2[r, jj]          # [10, 7]
                    nz = np.nonzero(wv)
                    for o, bn in zip(*nz):
                        wd[p, pp, NT1 + o,
                           py * 7 + bn + half * NBIN] += wv[o, bn]
            # --- L3: dense lhsT tiles 28..29
            j3 = np.full(N3, -1, np.int64)
            j3[:NR3] = idx3[ra]
            j3[NR3:2 * NR3] = idx3[rb]
            for half, r in ((0, ra), (1, rb)):
                for jj in range(NR3):
                    J = half * NR3 + jj
                    ch, pp = J // 128, J % 128
                    wv = lt3[r, jj]         # [49]
                    nz = np.nonzero(wv)[0]
                    for bn in nz:
                        wd[p, pp, NT1 + NT2 + ch, bn + half * NBIN] += wv[bn]
            # --- pack idx
            col = p * PAIR_ICOLS
            idxs[:, col:col + N0 // 16] = _pack_idx(j0)
            col += N0 // 16
            idxs[:, col:col + N1 // 16] = _pack_idx(j1)
            col += N1 // 16
            idxs[:, col:col + N2 // 16] = _pack_idx(j2)
            col += N2 // 16
            idxs[:, col:col + N3 // 16] = _pack_idx(j3)

        in_maps.append({
            "f0": feats[0][b], "f1": feats[1][b],
            "f2": feats[2][b], "f3": feats[3][b],
            "cst": cst, "idxs": idxs,
            "wd": wd.reshape(NPAIR, 128, NTD * NBIN2),
        })
    return in_maps


def _build_module():
    from concourse import bacc, tile
    from concourse.bass import mybir
    import concourse.bass as bass_mod

    F32 = mybir.dt.float32
    F16 = mybir.dt.float16
    I16 = mybir.dt.int16
    AP = bass_mod.AP

    nc = bacc.Bacc(None, target_bir_lowering=False, num_swdge_queues=4)
    f0 = nc.dram_tensor("f0", [F0_ROWS, C], F16, kind="ExternalInput")
    f1 = nc.dram_tensor("f1", [F1_ROWS, C], F16, kind="ExternalInput")
    f2 = nc.dram_tensor("f2", [F2_ROWS, C], F16, kind="ExternalInput")
    f3 = nc.dram_tensor("f3", [F3_ROWS, C], F16, kind="ExternalInput")
    cst = nc.dram_tensor("cst", [128, CST_COLS], F16, kind="ExternalInput")
    idxs = nc.dram_tensor("idxs", [128, IDX_COLS], I16, kind="ExternalInput")
    wd = nc.dram_tensor("wd", [NPAIR, 128, NTD * NBIN2], F16, kind="ExternalInput")
    out = nc.dram_tensor("out", [NPAIR, NBIN2, C], F32, kind="ExternalOutput")

    # strided views for strip gathers (strides/sizes in fp16 elements)
    f0v = AP(f0, 0, [[2 * C, (F0_ROWS - W0) // 2 + 1], [1, W0 * C]])
    f1v = AP(f1, 0, [[C, F1_ROWS - W1 + 1], [1, W1 * C]])
    f2v = AP(f2, 0, [[C, F2_ROWS - W2 + 1], [1, W2 * C]])
    f3v = AP(f3, 0, [[C, F3_ROWS], [1, C]])

    GATHERS = [
        # (src, nidx, chunks, elem_els, step_els, icols)
        (f0v, N0, CH0, W0 * C, 2 * C, N0 // 16),
        (f1v, N1, CH1, W1 * C, C, N1 // 16),
        (f2v, N2, CH2, W2 * C, C, N2 // 16),
        (f3v, N3, CH3, C, C, N3 // 16),
    ]

    with tile.TileContext(nc) as tc:
        with (
            tc.tile_pool(name="const", bufs=1) as constp,
            tc.tile_pool(name="g0p", bufs=2) as g0p,
            tc.tile_pool(name="g1p", bufs=2) as g1p,
            tc.tile_pool(name="g2p", bufs=2) as g2p,
            tc.tile_pool(name="g3p", bufs=2) as g3p,
            tc.tile_pool(name="w0p", bufs=2) as w0p,
            tc.tile_pool(name="wdp", bufs=2) as wdp,
            tc.tile_pool(name="accp", bufs=4, space="PSUM") as accp,
            tc.tile_pool(name="evp", bufs=3) as evp,
        ):
            cst_t = constp.tile([128, CST_COLS], F16)
            nc.sync.dma_start(cst_t[:], cst[:])
            idx_t = constp.tile([128, IDX_COLS], I16)
            nc.sync.dma_start(idx_t[:], idxs[:])

            gpools = [g0p, g1p, g2p, g3p]
            pat0_ap = cst_t[:, 0:PAT0_COLS]

            for p in range(NPAIR):
                gts = []
                col = p * PAIR_ICOLS
                for l, (src, nidx, chk, elem, step, icols) in enumerate(GATHERS):
                    gt = gpools[l].tile([128, chk, elem], F16, tag=f"g{l}")
                    if p < 2:
                        nc.vector.memset(gt[:], 0)
                    nc.gpsimd.dma_gather(
                        gt[:], src, idx_t[:, col:col + icols],
                        nidx, nidx, elem, elem_step=step, queue_num=l)
                    col += icols
                    gts.append(gt)

                # L0 weights: one broadcast DVE op
                w0t = w0p.tile([128, CH0, W0, NBIN2], F16, tag="w0")
                pat_b = (pat0_ap.rearrange("p (c b) -> p c b", c=CH0)
                         .unsqueeze(2).broadcast_to((128, CH0, W0, NBIN2)))
                so = PAT0_COLS + p * SCAL0_PER_PAIR
                scal_b = (cst_t[:, so:so + SCAL0_PER_PAIR]
                          .rearrange("p (c s) -> p c s", c=CH0)
                          .unsqueeze(3).broadcast_to((128, CH0, W0, NBIN2)))
                nc.vector.tensor_tensor(w0t[:], pat_b, scal_b,
                                        mybir.AluOpType.mult)

                wdt = wdp.tile([128, NTD * NBIN2], F16, tag="wd")
                nc.sync.dma_start(wdt[:], wd[p])

                acc = accp.tile([NBIN2, C], F32)
                n_mm = CH0 * W0 + NT1 + NT2 + NT3
                mi = 0
                for c in range(CH0):
                    for sl in range(W0):
                        nc.tensor.matmul(
                            acc[:], w0t[:, c, sl, :],
                            gts[0][:, c, sl * C:(sl + 1) * C],
                            start=(mi == 0), stop=(mi == n_mm - 1))
                        mi += 1
                for c in range(CH1):
                    for sl in range(W1):
                        t = c * W1 + sl
                        nc.tensor.matmul(
                            acc[:], wdt[:, t * NBIN2:(t + 1) * NBIN2],
                            gts[1][:, c, sl * C:(sl + 1) * C],
                            start=(mi == 0), stop=(mi == n_mm - 1))
                        mi += 1
                for sl in range(W2):
                    t = NT1 + sl
                    nc.tensor.matmul(
                        acc[:], wdt[:, t * NBIN2:(t + 1) * NBIN2],
                        gts[2][:, 0, sl * C:(sl + 1) * C],
                        start=(mi == 0), stop=(mi == n_mm - 1))
                    mi += 1
                for c in range(CH3):
                    t = NT1 + NT2 + c
                    nc.tensor.matmul(
                        acc[:], wdt[:, t * NBIN2:(t + 1) * NBIN2],
                        gts[3][:, c, :],
                        start=(mi == 0), stop=(mi == n_mm - 1))
                    mi += 1

                ev = evp.tile([NBIN2, C], F32, tag="ev")
                nc.scalar.copy(ev[:], acc[:])
                nc.sync.dma_start(out[p], ev[:])
    nc.finalize()
    return nc


def kernel(x0, x1, x2, x3, boxes):
    from concourse.bass_utils import run_bass_kernel_spmd
    in_maps = _host_prepare(x0, x1, x2, x3, boxes)
    if 'nc' not in _MODULE_CACHE:
        _MODULE_CACHE['nc'] = _build_module()
    nc = _MODULE_CACHE['nc']
    res = run_bass_kernel_spmd(nc, in_maps, list(range(8)))
    _MODULE_CACHE['last_result'] = res
    outs = [res.results[k]["out"] for k in range(8)]
    full = np.concatenate(outs, axis=0)            # [512, 98, 256]
    full = full.reshape(1024, NBIN, C).transpose(0, 2, 1)
    return np.ascontiguousarray(full).reshape(1024, C, POOLED, POOLED).astype(np.float32)


# revision 16
# speedup vs baseline: 2.7610x; 1.0422x over previous
"""Multi-level ROI Align (FPN pooler, 4 levels summed) on 8 Trainium2 cores.

v3: descriptor-minimized fp16 design. Shard ROIs across cores (core k:
batch k//4, 128 ROIs as 64 pairs). Host computes all gather indices and
bilinear weights from `boxes`; device does HBM strip-gathers (dma_gather,
one descriptor per multi-pixel strip) + fp16 matmuls accumulating both
ROIs of a pair into one PSUM tile [98, 256] (bins 0-48 = ROI a, 49-97 =
ROI b), evicted straight to DRAM as [pair, 98, 256] fp32; host transposes.

Per pair-of-ROIs:
  L0 (200x200, s=.25):  2x196 strips of 8px (even-aligned, idx=flat>>1),
      one strip per (rowsel, ysample, xbin); rank-1 weights
      (fixed one-hot bin pattern x per-strip scalar, built by one DVE op)
      -> 4 chunks x 8 slots = 32 matmuls
  L1 (100x100, s=.125): 2x112 strips of 9px, one per (rowsel, ysample,
      xbin-pair); host-baked dense lhsT -> 2 chunks x 9 slots = 18 mm
  L2 (50x50, s=.0625):  2x56 strips of 10px, one per (rowsel, ysample,
      xbin-quad); dense lhsT -> 1 chunk x 10 slots = 10 mm
  L3 (25x25, s=.03125): 2x100 px of the 10x10 bounding region; dense
      separable lhsT -> 2 chunks x 1 slot = 2 mm
"""
import sys
import numpy as np

sys.path.insert(0, '/opt/trn_rl_repo')

POOLED = 7
SAMP = 2
NBIN = 49
C = 256
IMG = 800.0

NROI_CORE = 128
NPAIR = 64

# per-level geometry
L0 = dict(H=200, W=200, scale=0.25)
L1 = dict(H=100, W=100, scale=0.125)
L2 = dict(H=50, W=50, scale=0.0625)
L3 = dict(H=25, W=25, scale=0.03125)

W0, W1, W2 = 8, 9, 10                  # strip widths (px)
NS0, NS1, NS2 = 196, 112, 56           # strips per ROI
N0, N1, N2 = 400, 224, 112             # padded pair nidx (%16, incl -1 tail)
CH0, CH1, CH2 = 4, 2, 1                # chunks per pair
CH3 = 5                                # L3 full-map chunks (625px -> 5x128)
NT1, NT2, NT3 = CH1 * W1, W2, CH3      # dense lhsT tiles per level
NTD = NT1 + NT2 + NT3                  # 18 + 10 + 5 = 33
NBIN2 = 2 * NBIN                       # 98
NBLK = NPAIR // 2                      # 2-pair blocks

F0_ROWS = 40004
F1_ROWS = 10000
F2_ROWS = 2500
F3_ROWS = 640

# cst fp16 column layout: L0 pattern [4ch, 98] then L0 scalars
PAT0_COLS = CH0 * NBIN2                        # 392
SCAL0_PER_PAIR = CH0 * W0                      # 32
CST_COLS = PAT0_COLS + NPAIR * SCAL0_PER_PAIR  # 392 + 2048

# idx int16 layout per pair: [L0 400][L1 224][L2 112] / 16
PAIR_ICOLS = (N0 + N1 + N2) // 16              # 46
IDX_COLS = NPAIR * PAIR_ICOLS

_MODULE_CACHE = {}


def _sample_meta(boxes_b, H, W, scale):
    """Per-ROI sample geometry in fp32, matching reference op order.
    boxes_b: [N, 4] fp32. Returns dict of [N,7,2] arrays."""
    f = np.float32
    b = boxes_b.astype(np.float32)
    x1 = b[:, 0] * f(scale)
    y1 = b[:, 1] * f(scale)
    x2 = b[:, 2] * f(scale)
    y2 = b[:, 3] * f(scale)
    rw = np.maximum(x2 - x1, f(1.0))
    rh = np.maximum(y2 - y1, f(1.0))
    bw = rw / f(POOLED)
    bh = rh / f(POOLED)
    g = (np.arange(POOLED, dtype=np.float32)[:, None]
         + (np.arange(SAMP, dtype=np.float32)[None, :] + f(0.5)) / f(SAMP))
    y = y1[:, None, None] + g[None] * bh[:, None, None]   # [N,7,2]
    x = x1[:, None, None] + g[None] * bw[:, None, None]
    masky = ((y >= f(-1.0)) & (y <= f(H))).astype(np.float32)
    maskx = ((x >= f(-1.0)) & (x <= f(W))).astype(np.float32)
    yc = np.clip(y, f(0.0), f(H - 1))
    xc = np.clip(x, f(0.0), f(W - 1))
    yl = np.floor(yc).astype(np.int64)
    xl = np.floor(xc).astype(np.int64)
    yh = np.minimum(yl + 1, H - 1)
    xh = np.minimum(xl + 1, W - 1)
    ly = (yc - yl.astype(np.float32)).astype(np.float32)
    lx = (xc - xl.astype(np.float32)).astype(np.float32)
    hy = (f(1.0) - ly).astype(np.float32)
    hx = (f(1.0) - lx).astype(np.float32)
    return dict(yl=yl, yh=yh, xl=xl, xh=xh, ly=ly, lx=lx, hy=hy, hx=hx,
                masky=masky, maskx=maskx, x=x, y=y)


def _strip_grid(meta, W, bins_per_strip, width, parity):
    """Build per-ROI strip indices and slot weights.

    Strips: (rowsel 2) x (ysample 14) x (xgroup ceil(7/b)).
    Returns idx [N, NS] int64 (pixel index of strip start, or start>>1 if
    parity), wslot [N, NS, width] fp32 (bilinear x-weights x y-weight x
    mask x 0.25), and bins [NS_xgroups arrays] for pattern building is
    implicit: each strip's samples' bins vary within the group -- handled
    by caller via per-sample info: also returns sample slot/bin arrays:
    contrib = (slotpos [N, NS, 7bins?..]) -- instead we return dense
    per-strip per-slot per-bin weights only when needed. For rank-1 (b=1)
    wslot is enough (all slots -> the strip's single bin).
    """
    N = meta['yl'].shape[0]
    f = np.float32
    ngrp = -(-POOLED // bins_per_strip)           # x-groups per row
    NS = 2 * 14 * ngrp
    # rows/yweights: [N, 2, 14]
    rows = np.stack([meta['yl'], meta['yh']], axis=1).reshape(N, 2, 14)
    wy = (np.stack([meta['hy'], meta['ly']], axis=1)
          * meta['masky'][:, None]).reshape(N, 2, 14).astype(np.float32)
    # x corners per sample: [N, 7, 2]
    xl, xh = meta['xl'], meta['xh']
    wxl = (meta['hx'] * meta['maskx']).astype(np.float32)
    wxh = (meta['lx'] * meta['maskx']).astype(np.float32)
    # group starts: min xl over samples in group -> [N, ngrp]
    xs = np.empty((N, ngrp), np.int64)
    for gi in range(ngrp):
        b0, b1 = gi * bins_per_strip, min((gi + 1) * bins_per_strip, POOLED)
        xs[:, gi] = xl[:, b0:b1, :].reshape(N, -1).min(axis=1)
    if parity:
        xs &= ~1
    xs = np.clip(xs, 0, W - width)
    # slot weights [N, ngrp, width] per (bin-in-group, sx): scatter
    wslot = np.zeros((N, ngrp, width, POOLED), np.float32)  # per-bin slots
    ridx = np.arange(N)[:, None, None]
    for gi in range(ngrp):
        b0, b1 = gi * bins_per_strip, min((gi + 1) * bins_per_strip, POOLED)
        for bx in range(b0, b1):
            for sx in range(SAMP):
                ol = xl[:, bx, sx] - xs[:, gi]
                oh = xh[:, bx, sx] - xs[:, gi]
                np.add.at(wslot, (np.arange(N), gi, ol, bx), wxl[:, bx, sx])
                np.add.at(wslot, (np.arange(N), gi, oh, bx), wxh[:, bx, sx])
    # combine with y: strips [N, 2, 14, ngrp]
    idx = (rows[:, :, :, None] * W + xs[:, None, None, :])
    if parity:
        idx >>= 1
    idx = idx.reshape(N, NS)
    # w [N, 2, 14, ngrp, width, 7] = wy * wslot * 0.25
    w = (wy[:, :, :, None, None, None]
         * wslot[:, None, None, :, :, :] * f(0.25))
    return idx, w.reshape(N, NS, width, POOLED)


def _build_l3_dense(meta):
    """L3: dense separable weights over the FULL 25x25 map (features live
    in SBUF on device). Returns lhsT [N, 640, 49] fp32 (px zero-padded)."""
    N = meta['yl'].shape[0]
    H = W = 25
    f = np.float32
    WY = np.zeros((N, H, POOLED), np.float32)
    WX = np.zeros((N, W, POOLED), np.float32)
    ridx = np.arange(N)[:, None, None]
    pidx = np.broadcast_to(np.arange(POOLED)[None, :, None], (N, POOLED, SAMP))
    np.add.at(WY, (ridx, meta['yl'], pidx),
              (f(0.5) * meta['hy'] * meta['masky']).astype(np.float32))
    np.add.at(WY, (ridx, meta['yh'], pidx),
              (f(0.5) * meta['ly'] * meta['masky']).astype(np.float32))
    np.add.at(WX, (ridx, meta['xl'], pidx),
              (f(0.5) * meta['hx'] * meta['maskx']).astype(np.float32))
    np.add.at(WX, (ridx, meta['xh'], pidx),
              (f(0.5) * meta['lx'] * meta['maskx']).astype(np.float32))
    lhsT = np.einsum('nap,nbq->nabpq', WY, WX).reshape(N, H * W, NBIN)
    out = np.zeros((N, CH3 * 128, NBIN), np.float32)
    out[:, :H * W] = lhsT
    return out


def _pack_idx(idx_flat):
    """[n] int -> [128, n//16] int16: wrap 16 partitions, replicate 8x."""
    n = idx_flat.shape[0]
    arr = idx_flat.reshape(n // 16, 16).T            # [16, cols]
    arr = np.broadcast_to(arr[None], (8, 16, n // 16)).reshape(128, n // 16)
    return arr.astype(np.int16)


def _l0_pattern():
    """Fixed one-hot [128, CH0, 98] fp16: J = c*128+p -> bin."""
    pat = np.zeros((CH0, 128, NBIN2), np.float16)
    for J in range(2 * NS0):
        half, j = J // NS0, J % NS0
        bx = j % POOLED
        t = (j // POOLED) % 14
        py = t // 2
        pat[J // 128, J % 128, py * 7 + bx + half * NBIN] = 1.0
    return pat.transpose(1, 0, 2)                    # [128, CH0, 98]


def _strip_scatter(wd, w, half, ns, ngrp, width, t_base, chunks_w):
    """Scatter per-ROI strip weights [NROI_CORE, ns, width, 7] into wd
    [NPAIR, 128, NTD, 98]. half: 0/1 (roi parity within pair)."""
    jj = np.arange(ns)
    J = half * ns + jj
    ch, pp = J // 128, J % 128
    py = ((jj // ngrp) % 14) // 2
    t = t_base + ch[:, None] * chunks_w + np.arange(width)[None, :]  # [ns, width]
    bn = py[:, None] * 7 + np.arange(POOLED)[None, :] + half * NBIN  # [ns, 7]
    rois = np.arange(half, NROI_CORE, 2)
    pair_i = np.broadcast_to((rois // 2)[:, None, None, None],
                             (NPAIR, ns, width, POOLED))
    pp_i = np.broadcast_to(pp[None, :, None, None], pair_i.shape)
    t_i = np.broadcast_to(t[None, :, :, None], pair_i.shape)
    bn_i = np.broadcast_to(bn[None, :, None, :], pair_i.shape)
    np.add.at(wd, (pair_i, pp_i, t_i, bn_i), w[rois])


def _host_prepare(x0, x1, x2, x3, boxes):
    """Build all per-core input tensors. Returns list of 8 dicts."""
    B = boxes.shape[0]
    feats = []
    for arr, lv, rows in ((x0, L0, F0_ROWS), (x1, L1, F1_ROWS),
                          (x2, L2, F2_ROWS), (x3, L3, F3_ROWS)):
        f = np.zeros((B, rows, C), np.float16)
        hw = lv['H'] * lv['W']
        f[:, :hw] = np.ascontiguousarray(
            np.transpose(np.asarray(arr, np.float32), (0, 2, 3, 1))
        ).reshape(B, hw, C).astype(np.float16)
        feats.append(f)
    # f3 rearranged for SBUF residency: [128, CH3, C], px = ch*128 + p
    f3s = np.ascontiguousarray(
        feats[3].reshape(B, CH3, 128, C).transpose(0, 2, 1, 3))

    pat0 = _l0_pattern()

    per_batch = []
    for b in range(B):
        bb = np.asarray(boxes[b], np.float32)
        m0 = _sample_meta(bb, L0['H'], L0['W'], L0['scale'])
        m1 = _sample_meta(bb, L1['H'], L1['W'], L1['scale'])
        m2 = _sample_meta(bb, L2['H'], L2['W'], L2['scale'])
        m3 = _sample_meta(bb, L3['H'], L3['W'], L3['scale'])
        idx0, w0 = _strip_grid(m0, L0['W'], 1, W0, parity=True)   # [N,196],[N,196,8,7]
        idx1, w1 = _strip_grid(m1, L1['W'], 2, W1, parity=False)  # [N,112],[N,112,9,7]
        idx2, w2 = _strip_grid(m2, L2['W'], 4, W2, parity=False)  # [N,56],[N,56,10,7]
        lt3 = _build_l3_dense(m3)                                 # [N,640,49]
        per_batch.append((idx0, w0, idx1, w1, idx2, w2, lt3))

    in_maps = []
    for k in range(8):
        b = k // 4
        s = (k % 4) * NROI_CORE
        idx0, w0, idx1, w1, idx2, w2, lt3 = per_batch[b]
        sl = slice(s, s + NROI_CORE)

        cst = np.zeros((128, CST_COLS), np.float16)
        cst[:, :PAT0_COLS] = pat0.reshape(128, -1)

        idxs = np.zeros((128, IDX_COLS), np.int16)
        wd = np.zeros((NPAIR, 128, NTD, NBIN2), np.float32)

        # dense lhsT: L1 tiles 0..17, L2 18..27 (vectorized scatter)
        _strip_scatter(wd, w1[sl], 0, NS1, 4, W1, 0, W1)
        _strip_scatter(wd, w1[sl], 1, NS1, 4, W1, 0, W1)
        _strip_scatter(wd, w2[sl], 0, NS2, 2, W2, NT1, W2)
        _strip_scatter(wd, w2[sl], 1, NS2, 2, W2, NT1, W2)
        # L3 tiles 28..32: dense full-map separable weights
        lt3c = lt3[sl].reshape(NPAIR, 2, CH3, 128, NBIN)
        wd[:, :, NT1 + NT2:, :NBIN] = lt3c[:, 0].transpose(0, 2, 1, 3)
        wd[:, :, NT1 + NT2:, NBIN:] = lt3c[:, 1].transpose(0, 2, 1, 3)

        for p in range(NPAIR):
            ra, rb = s + 2 * p, s + 2 * p + 1
            # --- L0: scal table + idx
            j0 = np.full(N0, -1, np.int64)
            j0[:NS0] = idx0[ra]
            j0[NS0:2 * NS0] = idx0[rb]
            sc = np.zeros((CH0 * 128, W0), np.float32)
            wpair = np.concatenate([w0[ra], w0[rb]], axis=0)  # [392, 8, 7]
            bins_x = np.tile(np.arange(NS0) % POOLED, 2)
            sc[:2 * NS0] = wpair[np.arange(2 * NS0), :, bins_x]
            cst[:, PAT0_COLS + p * SCAL0_PER_PAIR:
                PAT0_COLS + (p + 1) * SCAL0_PER_PAIR] = (
                sc.reshape(CH0, 128, W0).transpose(1, 0, 2)
                .reshape(128, SCAL0_PER_PAIR).astype(np.float16))
            # --- idx
            j1 = np.empty(N1, np.int64)
            j1[:NS1] = idx1[ra]
            j1[NS1:] = idx1[rb]
            j2 = np.empty(N2, np.int64)
            j2[:NS2] = idx2[ra]
            j2[NS2:] = idx2[rb]
            col = p * PAIR_ICOLS
            idxs[:, col:col + N0 // 16] = _pack_idx(j0)
            col += N0 // 16
            idxs[:, col:col + N1 // 16] = _pack_idx(j1)
            col += N1 // 16
            idxs[:, col:col + N2 // 16] = _pack_idx(j2)

        in_maps.append({
            "f0": feats[0][b], "f1": feats[1][b],
            "f2": feats[2][b], "f3s": f3s[b],
            "cst": cst, "idxs": idxs,
            "wd": wd.astype(np.float16).reshape(
                NBLK, 2, 128, NTD * NBIN2).transpose(0, 2, 1, 3).reshape(
                NBLK, 128, 2 * NTD * NBIN2),
        })
    return in_maps


def _build_module():
    from concourse import bacc, tile
    from concourse.bass import mybir
    import concourse.bass as bass_mod

    F32 = mybir.dt.float32
    F16 = mybir.dt.float16
    I16 = mybir.dt.int16
    AP = bass_mod.AP

    nc = bacc.Bacc(None, target_bir_lowering=False, num_swdge_queues=4)
    f0 = nc.dram_tensor("f0", [F0_ROWS, C], F16, kind="ExternalInput")
    f1 = nc.dram_tensor("f1", [F1_ROWS, C], F16, kind="ExternalInput")
    f2 = nc.dram_tensor("f2", [F2_ROWS, C], F16, kind="ExternalInput")
    f3s = nc.dram_tensor("f3s", [128, CH3 * C], F16, kind="ExternalInput")
    cst = nc.dram_tensor("cst", [128, CST_COLS], F16, kind="ExternalInput")
    idxs = nc.dram_tensor("idxs", [128, IDX_COLS], I16, kind="ExternalInput")
    wd = nc.dram_tensor("wd", [NBLK, 128, 2 * NTD * NBIN2], F16,
                        kind="ExternalInput")
    out = nc.dram_tensor("out", [NBIN2, NPAIR, C], F32, kind="ExternalOutput")

    # strided views for strip gathers (strides/sizes in fp16 elements)
    f0v = AP(f0, 0, [[2 * C, (F0_ROWS - W0) // 2 + 1], [1, W0 * C]])
    f1v = AP(f1, 0, [[C, F1_ROWS - W1 + 1], [1, W1 * C]])
    f2v = AP(f2, 0, [[C, F2_ROWS - W2 + 1], [1, W2 * C]])

    GATHERS = [
        # (src, nidx, chunks, elem_els, step_els, icols)
        (f0v, N0, CH0, W0 * C, 2 * C, N0 // 16),
        (f1v, N1, CH1, W1 * C, C, N1 // 16),
        (f2v, N2, CH2, W2 * C, C, N2 // 16),
    ]

    with tile.TileContext(nc) as tc:
        with (
            tc.tile_pool(name="const", bufs=1) as constp,
            tc.tile_pool(name="g0p", bufs=2) as g0p,
            tc.tile_pool(name="g1p", bufs=2) as g1p,
            tc.tile_pool(name="g2p", bufs=2) as g2p,
            tc.tile_pool(name="w0p", bufs=2) as w0p,
            tc.tile_pool(name="wdp", bufs=2) as wdp,
            tc.tile_pool(name="accp", bufs=4, space="PSUM") as accp,
            tc.tile_pool(name="evp", bufs=2) as evp,
        ):
            cst_t = constp.tile([128, CST_COLS], F16)
            nc.sync.dma_start(cst_t[:], cst[:])
            idx_t = constp.tile([128, IDX_COLS], I16)
            nc.sync.dma_start(idx_t[:], idxs[:])
            f3t = constp.tile([128, CH3, C], F16)
            nc.sync.dma_start(f3t[:], f3s.rearrange("p (h c) -> p h c", h=CH3))

            gpools = [g0p, g1p, g2p]
            pat0_ap = cst_t[:, 0:PAT0_COLS]
            n_mm = CH0 * W0 + NT1 + NT2 + NT3

            for blk in range(NBLK):
                wdt = wdp.tile([128, 2, NTD * NBIN2], F16, tag="wd")
                nc.sync.dma_start(wdt[:], wd[blk].rearrange(
                    "p (i t) -> p i t", i=2))
                ev = evp.tile([NBIN2, 2, C], F32, tag="ev")
                for half in range(2):
                    p = blk * 2 + half
                    gts = []
                    col = p * PAIR_ICOLS
                    for l, (src, nidx, chk, elem, step, icols) in enumerate(GATHERS):
                        gt = gpools[l].tile([128, chk, elem], F16, tag=f"g{l}")
                        if p < 2:
                            nc.vector.memset(gt[:], 0)
                        nc.gpsimd.dma_gather(
                            gt[:], src, idx_t[:, col:col + icols],
                            nidx, nidx, elem, elem_step=step, queue_num=l)
                        col += icols
                        gts.append(gt)

                    # L0 weights: one broadcast DVE op
                    w0t = w0p.tile([128, CH0, W0, NBIN2], F16, tag="w0")
                    pat_b = (pat0_ap.rearrange("p (c b) -> p c b", c=CH0)
                             .unsqueeze(2).broadcast_to((128, CH0, W0, NBIN2)))
                    so = PAT0_COLS + p * SCAL0_PER_PAIR
                    scal_b = (cst_t[:, so:so + SCAL0_PER_PAIR]
                              .rearrange("p (c s) -> p c s", c=CH0)
                              .unsqueeze(3).broadcast_to((128, CH0, W0, NBIN2)))
                    nc.vector.tensor_tensor(w0t[:], pat_b, scal_b,
                                            mybir.AluOpType.mult)

                    acc = accp.tile([NBIN2, C], F32)
                    mi = 0
                    for c in range(CH0):
                        for sl in range(W0):
                            nc.tensor.matmul(
                                acc[:], w0t[:, c, sl, :],
                                gts[0][:, c, sl * C:(sl + 1) * C],
                                start=(mi == 0), stop=(mi == n_mm - 1))
                            mi += 1
                    for c in range(CH1):
                        for sl in range(W1):
                            t = c * W1 + sl
                            nc.tensor.matmul(
                                acc[:], wdt[:, half, t * NBIN2:(t + 1) * NBIN2],
                                gts[1][:, c, sl * C:(sl + 1) * C],
                                start=(mi == 0), stop=(mi == n_mm - 1))
                            mi += 1
                    for sl in range(W2):
                        t = NT1 + sl
                        nc.tensor.matmul(
                            acc[:], wdt[:, half, t * NBIN2:(t + 1) * NBIN2],
                            gts[2][:, 0, sl * C:(sl + 1) * C],
                            start=(mi == 0), stop=(mi == n_mm - 1))
                        mi += 1
                    for c in range(CH3):
                        t = NT1 + NT2 + c
                        nc.tensor.matmul(
                            acc[:], wdt[:, half, t * NBIN2:(t + 1) * NBIN2],
                            f3t[:, c, :],
                            start=(mi == 0), stop=(mi == n_mm - 1))
                        mi += 1

                    nc.scalar.copy(ev[:, half, :], acc[:])
                nc.sync.dma_start(out[:, 2 * blk:2 * blk + 2, :], ev[:])
    nc.finalize()
    return nc


def kernel(x0, x1, x2, x3, boxes):
    from concourse.bass_utils import run_bass_kernel_spmd
    in_maps = _host_prepare(x0, x1, x2, x3, boxes)
    if 'nc' not in _MODULE_CACHE:
        _MODULE_CACHE['nc'] = _build_module()
    nc = _MODULE_CACHE['nc']
    res = run_bass_kernel_spmd(nc, in_maps, list(range(8)))
    _MODULE_CACHE['last_result'] = res
    # per-core out is [98, 64, 256] bin-major: bin2 = half*49+bin
    parts = []
    for k in range(8):
        o = res.results[k]["out"].reshape(2, NBIN, NPAIR, C)
        parts.append(np.ascontiguousarray(
            o.transpose(2, 0, 3, 1)).reshape(NROI_CORE, C, NBIN))
    full = np.concatenate(parts, axis=0)           # [1024, 256, 49]
    return full.reshape(1024, C, POOLED, POOLED).astype(np.float32)
